# revision 60
# baseline (speedup 1.0000x reference)
"""Bass/Trainium2 kernel for nn_DocRelPrompt.

Reference computation (B=64, L=512, H=768, HEAD=64, N_PROMPTS=10, N_LBL=2):
    rel2 = stack([1-r, r], 1)                   # (B, 2)
    hidden_rel = rel2 @ label_prompts           # (B, H)
    Q  = prompts @ ref_qw.T + ref_qb            # (10, HEAD)
    K  = hid @ ref_kw.T + ref_kb                # (B, L, HEAD)
    scores[b,n] = mean_l(Q[n] . K[b,l]) / 8
                = (hsum[b] . (Q@ref_kw)[n] / (512*8)) + (Q[n].ref_kb)/8
    gate = sigmoid(scores)                      # (B, 10)
    doc  = prompts[None] * gate[..., None]      # (B, 10, H)
    out  = concat([doc, hid + hidden_rel[:,None,:]], axis=1)   # (B, 522, H)

(The `_rel_prompts` branch of the reference is computed but unused, so it is
skipped entirely.)

Sharding: pure data-parallel over batch, 8 cores x 8 batches.  The tiny
prompt/weight tensors are folded on the host into W2s (768,10, bf16) and
c2 (10,) and replicated; the label-prompt vectors arrive partition-broadcast
so each per-batch hidden_rel row is a single DVE op.

Device work per core, per batch (DMA-bound; ~26 MB HBM traffic total):
  - two half-tile hid loads (128, 2, 768) on the SP HWDGE ring (the first
    two batches are issued ahead of the const loads — the ring is FIFO);
  - ACT casts a bf16 shadow (fp32 stationaries load ~6x slower and fp32
    matmuls double-pump; the sigmoid-gate path tolerates bf16 easily);
  - PE: hsum[b] (1, 768) via ones-stationary matmuls PSUM-accumulated over
    L-tiles, ACT downcast, then 6 PE transposes build hsumT columns (bf16,
    pad dim keeps each column 4-byte aligned in PSUM);
  - per-batch gate tail: 6 bf16 matmuls accumulate scores[:, b], ACT
    sigmoid(+c2), DVE tensor_scalar doc row, 30 KB doc DMA — all pipelined
    so only the last batch's chain trails the stream;
  - DVE: rel row = db*r_b + lp0b (scalar_tensor_tensor), then per half-tile
    an in-place tensor_tensor body = hid + rel and a 0.75 MB body DMA on
    the ACT HWDGE ring (so outs never queue behind the next in-load).
"""

import numpy as np

B, L, H, HEAD, NPR, NLBL = 64, 512, 768, 64, 10, 2
NCORES = 8
BLOC = B // NCORES          # 8 batches per core
LT = L // 128               # 4 L-tiles of 128 partitions
HC = H // 128               # 6 H-chunks of 128

_CACHE = {}


def _build_module():
    from contextlib import ExitStack

    import concourse.bacc as bacc
    import concourse.mybir as mybir
    from concourse.tile import TileContext

    dt = mybir.dt.float32
    bf = mybir.dt.bfloat16
    ADD = mybir.AluOpType.add

    # Bacc (not raw Bass): its compile() legalizes sync waits — TRN2
    # instructions carry at most one wait, extras become event-sem waits.
    nc = bacc.Bacc("TRN2", target_bir_lowering=False, debug=False)
    hid = nc.dram_tensor("hid", [BLOC, L, H], dt, kind="ExternalInput")
    # fp32 constants packed into two tensors: one DMA dispatch each (the SP
    # HWDGE ring is FIFO; every extra dispatch delays the hid stream ~0.65us)
    constf = nc.dram_tensor("constf", [128, 2 * H + BLOC], dt, kind="ExternalInput")
    constp = nc.dram_tensor("constp", [NPR, H + 1], dt, kind="ExternalInput")
    w2st = nc.dram_tensor("w2st", [128, HC * NPR], bf, kind="ExternalInput")
    out = nc.dram_tensor("out", [BLOC, NPR + L, H], dt, kind="ExternalOutput")

    hid_r = hid[:].rearrange("b (t p) h -> b p t h", p=128)
    body_r = out[:, NPR:, :].rearrange("b (t p) h -> b p t h", p=128)

    with TileContext(nc) as tc, ExitStack() as ctx:
        const = ctx.enter_context(tc.tile_pool(name="const", bufs=1))
        hidp = ctx.enter_context(tc.tile_pool(name="hidp", bufs=6))
        bfp = ctx.enter_context(tc.tile_pool(name="bfp", bufs=4))
        relp = ctx.enter_context(tc.tile_pool(name="relp", bufs=3))
        hsp = ctx.enter_context(tc.tile_pool(name="hsp", bufs=2, space="PSUM"))
        hsbp = ctx.enter_context(tc.tile_pool(name="hsbp", bufs=2))
        sump = ctx.enter_context(tc.tile_pool(name="sump", bufs=1, space="PSUM"))
        scop = ctx.enter_context(tc.tile_pool(name="scop", bufs=2, space="PSUM"))
        warmp = ctx.enter_context(tc.tile_pool(name="warmp", bufs=1, space="PSUM"))
        small = ctx.enter_context(tc.tile_pool(name="small", bufs=1))

        ones_bf = const.tile([128, 1], bf)
        nc.vector.memset(ones_bf[:], 1.0)

        # issue the first two hid loads BEFORE the const loads — the SP
        # HWDGE ring is FIFO, and ~5us of const dispatches would otherwise
        # delay the first big transfer
        t_ins = []
        for b in range(2):
            t_in = hidp.tile([128, LT, H], dt, tag="hid")
            nc.sync.dma_start(t_in[:], hid_r[b])
            t_ins.append(t_in)

        w2st_sb = const.tile([128, HC * NPR], bf)
        nc.sync.dma_start(w2st_sb[:], w2st[:])
        constf_sb = const.tile([128, 2 * H + BLOC], dt)
        nc.sync.dma_start(constf_sb[:], constf[:])
        constp_sb = const.tile([NPR, H + 1], dt)
        nc.sync.dma_start(constp_sb[:], constp[:])
        lp0b_sb = constf_sb[:, 0:H]
        db_sb = constf_sb[:, H : 2 * H]
        rbc_sb = constf_sb[:, 2 * H : 2 * H + BLOC]
        prom_sb = constp_sb[:, 0:H]
        c2_sb = constp_sb[:, H : H + 1]

        # Warm-up matmuls: sync the PE against the DVE memset and the w2st
        # DMA one dependency at a time — matmuls tolerate few sync waits.
        scrap_ps = warmp.tile([128, 1], dt)
        nc.tensor.matmul(scrap_ps[0:1, :], ones_bf[:], ones_bf[:],
                         start=True, stop=True)
        nc.tensor.matmul(scrap_ps[0:NPR, :], w2st_sb[:, 0:NPR], ones_bf[:],
                         start=True, stop=True)
        # preload the sigmoid table during boot (1.3us ACT_TABLE_LOAD that
        # would otherwise land on the critical tail)
        sig_warm = small.tile([1, 1], dt)
        nc.scalar.activation(sig_warm[:], ones_bf[0:1, 0:1],
                             func=mybir.ActivationFunctionType.Sigmoid)

        # column c*BLOC+b = hsumT chunk; trailing pad dim keeps each bf16
        # transpose output column on a 4-byte PSUM boundary
        hsumT_ps = sump.tile([128, HC * BLOC, 2], bf)

        for b in range(BLOC):
            if b < 2:
                t_in = t_ins[b]
            else:
                t_in = hidp.tile([128, LT, H], dt, tag="hid")
                # half-tile loads: downstream compute starts ~2us earlier
                # and the outbound stream interleaves more smoothly
                nc.sync.dma_start(t_in[:, 0:2], hid_r[b][:, 0:2])
                nc.sync.dma_start(t_in[:, 2:4], hid_r[b][:, 2:4])

            # bf16 shadow copy for the PE column sums (fp32 stationary loads
            # are ~6x slower and fp32 matmuls double-pump; the gate path
            # tolerates bf16 easily).  ACT, not GpSimd: the GpSimd CAST runs
            # ~5x slower and steals SBUF ports from the DVE.
            t_bf = bfp.tile([128, LT, H], bf, tag="hidbf")
            nc.scalar.copy(t_bf[:, 0:2], t_in[:, 0:2])
            nc.scalar.copy(t_bf[:, 2:4], t_in[:, 2:4])

            # stage 1a: hsum (1, 768) = sum over (t, p) via ones-stationary
            # matmuls (stationary loaded once, moving = bf16 rows; PSUM
            # accumulation over the 4 L-tiles; split 512/256 on bank edge)
            hs_ps = hsp.tile([1, H], dt, tag="hs")
            for t in range(LT):
                nc.tensor.matmul(hs_ps[0:1, 0:512], ones_bf[:],
                                 t_bf[:, t, 0:512],
                                 start=(t == 0), stop=(t == LT - 1))
                nc.tensor.matmul(hs_ps[0:1, 512:H], ones_bf[:],
                                 t_bf[:, t, 512:H],
                                 start=(t == 0), stop=(t == LT - 1))

            # stage 1b: downcast hsum, transpose 128-chunks onto partitions
            hs_bf = hsbp.tile([1, H], bf, tag="hsbf")
            nc.scalar.copy(hs_bf[:], hs_ps[:])
            for c in range(HC):
                col = c * BLOC + b
                nc.tensor.transpose(
                    hsumT_ps[:, col, 0:1],
                    hs_bf[0:1, c * 128 : (c + 1) * 128],
                    ones_bf[0:1, 0:1],
                )

            # per-batch gate pipeline — score columns are independent, so
            # gate/doc/DMA for batch b complete long before the kernel tail
            hsT_b = hsbp.tile([128, HC], bf, tag="hstb")
            nc.scalar.copy(hsT_b[:], hsumT_ps[:, b :: BLOC, 0])
            score_b = scop.tile([NPR, 1], dt, tag="scoreb")
            for c in range(HC):
                nc.tensor.matmul(
                    score_b[:], w2st_sb[:, c * NPR : (c + 1) * NPR],
                    hsT_b[:, c : c + 1],
                    start=(c == 0), stop=(c == HC - 1),
                )
            gate_b = hsbp.tile([NPR, 1], dt, tag="gateb")
            nc.scalar.activation(
                gate_b[:], score_b[:],
                func=mybir.ActivationFunctionType.Sigmoid,
                bias=c2_sb, scale=1.0,
            )
            doc_b = hsbp.tile([NPR, H], dt, tag="docb")
            nc.vector.tensor_scalar(
                doc_b[:], prom_sb, gate_b[:, 0:1], None,
                mybir.AluOpType.mult,
            )
            nc.scalar.dma_start(out[b, 0:NPR, :], doc_b[:])

            # rel[b] = lp0 + r_b * (lp1 - lp0), already partition-broadcast on
            # the host; r_b enters as a per-partition scalar (DVE one op).
            rel_t = relp.tile([128, H], dt, tag="relsb")
            nc.vector.scalar_tensor_tensor(
                rel_t[:], db_sb, rbc_sb[:, b : b + 1], lp0b_sb,
                mybir.AluOpType.mult, ADD,
            )

            # body = hid + rel (in place, free-dim broadcast of rel over
            # L-tiles), in halves so each outbound half-DMA starts as soon as
            # its add lands.  out-DMAs ride the ACT HWDGE ring so they don't
            # queue behind the next batch's in-DMA on the SP ring.
            for hlf in range(2):
                sl = slice(2 * hlf, 2 * hlf + 2)
                nc.vector.tensor_tensor(
                    t_in[:, sl], t_in[:, sl],
                    rel_t[:, None, :].broadcast_to([128, 2, H]),
                    ADD,
                )
                nc.scalar.dma_start(body_r[b][:, sl], t_in[:, sl])

    nc.compile()
    return nc


def _host_fold(relevance, prompts, label_prompts, qw, qb, kw, kb):
    """Fold the tiny projection weights on the host.

    scores[b, n] = hsum[b] . W2s[:, n] + c2[n], with W2s/c2 absorbing the
    1/L mean pooling and the 1/sqrt(HEAD) scaling.
    """
    q = prompts.astype(np.float64) @ qw.astype(np.float64).T + qb.astype(np.float64)
    w2 = q @ kw.astype(np.float64)                               # (10, H)
    w2s = (w2.T / (L * np.sqrt(HEAD))).astype(np.float32)        # (H, 10)
    c2 = ((q @ kb.astype(np.float64)) / np.sqrt(HEAD)).astype(np.float32)  # (10,)
    # device layout: (128, HC*NPR), free index = c*NPR + n for h = c*128 + p
    import ml_dtypes

    w2st = np.ascontiguousarray(
        w2s.reshape(HC, 128, NPR).transpose(1, 0, 2).reshape(128, HC * NPR)
    ).astype(ml_dtypes.bfloat16)
    return w2st, c2.reshape(NPR, 1)


def _prepare_in_maps(
    relevance, hidden_states_src, prompts, label_prompts,
    ref_qw, ref_qb, ref_kw, ref_kb, **_unused,
):
    relevance = np.asarray(relevance, dtype=np.float32)
    hidden_states_src = np.ascontiguousarray(np.asarray(hidden_states_src, dtype=np.float32))
    prompts = np.ascontiguousarray(np.asarray(prompts, dtype=np.float32))
    label_prompts = np.asarray(label_prompts, dtype=np.float32)

    w2st, c2 = _host_fold(
        relevance, prompts, label_prompts,
        np.asarray(ref_qw, np.float32), np.asarray(ref_qb, np.float32),
        np.asarray(ref_kw, np.float32), np.asarray(ref_kb, np.float32),
    )
    dvec = label_prompts[1] - label_prompts[0]
    constp = np.concatenate([prompts, c2.reshape(NPR, 1)], axis=1)  # (10, 769)

    in_maps = []
    for core in range(NCORES):
        sl = slice(core * BLOC, (core + 1) * BLOC)
        constf = np.empty((128, 2 * H + BLOC), np.float32)
        constf[:, 0:H] = label_prompts[0]
        constf[:, H : 2 * H] = dvec
        constf[:, 2 * H :] = relevance[sl]
        in_maps.append(
            {
                "hid": np.ascontiguousarray(hidden_states_src[sl]),
                "constf": constf,
                "constp": np.ascontiguousarray(constp),
                "w2st": w2st,
            }
        )
    return in_maps


def _get_module():
    if "nc" not in _CACHE:
        _CACHE["nc"] = _build_module()
    return _CACHE["nc"]


def kernel(**inputs):
    from concourse.bass_utils import run_bass_kernel_spmd

    nc = _get_module()
    in_maps = _prepare_in_maps(**inputs)
    res = run_bass_kernel_spmd(nc, in_maps, list(range(NCORES)))
    return np.concatenate([res.results[c]["out"] for c in range(NCORES)], axis=0)


# revision 61
# speedup vs baseline: 1.0087x; 1.0087x over previous
"""Bass/Trainium2 kernel for nn_DocRelPrompt.

Reference computation (B=64, L=512, H=768, HEAD=64, N_PROMPTS=10, N_LBL=2):
    rel2 = stack([1-r, r], 1)                   # (B, 2)
    hidden_rel = rel2 @ label_prompts           # (B, H)
    Q  = prompts @ ref_qw.T + ref_qb            # (10, HEAD)
    K  = hid @ ref_kw.T + ref_kb                # (B, L, HEAD)
    scores[b,n] = mean_l(Q[n] . K[b,l]) / 8
                = (hsum[b] . (Q@ref_kw)[n] / (512*8)) + (Q[n].ref_kb)/8
    gate = sigmoid(scores)                      # (B, 10)
    doc  = prompts[None] * gate[..., None]      # (B, 10, H)
    out  = concat([doc, hid + hidden_rel[:,None,:]], axis=1)   # (B, 522, H)

(The `_rel_prompts` branch of the reference is computed but unused, so it is
skipped entirely.)

Sharding: pure data-parallel over batch, 8 cores x 8 batches.  The tiny
prompt/weight tensors are folded on the host into W2s (768,10, bf16) and
c2 (10,) and replicated; the label-prompt vectors arrive partition-broadcast
so each per-batch hidden_rel row is a single DVE op.

Device work per core, per batch (DMA-bound; ~26 MB HBM traffic total):
  - two half-tile hid loads (128, 2, 768) on the SP HWDGE ring (the first
    two batches are issued ahead of the const loads — the ring is FIFO);
  - ACT casts a bf16 shadow (fp32 stationaries load ~6x slower and fp32
    matmuls double-pump; the sigmoid-gate path tolerates bf16 easily);
  - PE: hsum[b] (1, 768) via ones-stationary matmuls PSUM-accumulated over
    L-tiles, ACT downcast, then 6 PE transposes build hsumT columns (bf16,
    pad dim keeps each column 4-byte aligned in PSUM);
  - per-batch gate tail: 6 bf16 matmuls accumulate scores[:, b], ACT
    sigmoid(+c2), DVE tensor_scalar doc row, 30 KB doc DMA — all pipelined
    so only the last batch's chain trails the stream;
  - DVE: rel row = db*r_b + lp0b (scalar_tensor_tensor), then per half-tile
    an in-place tensor_tensor body = hid + rel and a 0.75 MB body DMA on
    the ACT HWDGE ring (so outs never queue behind the next in-load).
"""

import numpy as np

B, L, H, HEAD, NPR, NLBL = 64, 512, 768, 64, 10, 2
NCORES = 8
BLOC = B // NCORES          # 8 batches per core
LT = L // 128               # 4 L-tiles of 128 partitions
HC = H // 128               # 6 H-chunks of 128

_CACHE = {}


def _build_module():
    from contextlib import ExitStack

    import concourse.bacc as bacc
    import concourse.mybir as mybir
    from concourse.tile import TileContext

    dt = mybir.dt.float32
    bf = mybir.dt.bfloat16
    ADD = mybir.AluOpType.add

    # Bacc (not raw Bass): its compile() legalizes sync waits — TRN2
    # instructions carry at most one wait, extras become event-sem waits.
    nc = bacc.Bacc("TRN2", target_bir_lowering=False, debug=False)
    hid = nc.dram_tensor("hid", [BLOC, L, H], dt, kind="ExternalInput")
    # fp32 constants packed into two tensors: one DMA dispatch each (the SP
    # HWDGE ring is FIFO; every extra dispatch delays the hid stream ~0.65us)
    constf = nc.dram_tensor("constf", [128, 2 * H + BLOC], dt, kind="ExternalInput")
    constp = nc.dram_tensor("constp", [NPR, H + 1], dt, kind="ExternalInput")
    w2st = nc.dram_tensor("w2st", [128, HC * NPR], bf, kind="ExternalInput")
    out = nc.dram_tensor("out", [BLOC, NPR + L, H], dt, kind="ExternalOutput")

    hid_r = hid[:].rearrange("b (t p) h -> b p t h", p=128)
    body_r = out[:, NPR:, :].rearrange("b (t p) h -> b p t h", p=128)

    with TileContext(nc) as tc, ExitStack() as ctx:
        const = ctx.enter_context(tc.tile_pool(name="const", bufs=1))
        hidp = ctx.enter_context(tc.tile_pool(name="hidp", bufs=6))
        bfp = ctx.enter_context(tc.tile_pool(name="bfp", bufs=4))
        relp = ctx.enter_context(tc.tile_pool(name="relp", bufs=3))
        hsp = ctx.enter_context(tc.tile_pool(name="hsp", bufs=2, space="PSUM"))
        hsbp = ctx.enter_context(tc.tile_pool(name="hsbp", bufs=2))
        sump = ctx.enter_context(tc.tile_pool(name="sump", bufs=1, space="PSUM"))
        scop = ctx.enter_context(tc.tile_pool(name="scop", bufs=2, space="PSUM"))
        warmp = ctx.enter_context(tc.tile_pool(name="warmp", bufs=1, space="PSUM"))
        small = ctx.enter_context(tc.tile_pool(name="small", bufs=1))

        ones_bf = const.tile([128, 1], bf)
        nc.vector.memset(ones_bf[:], 1.0)

        # issue the first two hid loads BEFORE the const loads — the SP
        # HWDGE ring is FIFO, and const dispatches would otherwise delay the
        # first big transfer
        t_ins = []
        for b in range(2):
            t_in = hidp.tile([128, LT, H], dt, tag="hid")
            nc.sync.dma_start(t_in[:, 0:2], hid_r[b][:, 0:2])
            nc.sync.dma_start(t_in[:, 2:4], hid_r[b][:, 2:4])
            t_ins.append(t_in)

        # constf feeds the rel path (needed by the first body add) -> SP ring
        # right after the prefetches; w2st/constp are only needed by the
        # slack gate path -> ACT ring, which is empty this early
        constf_sb = const.tile([128, 2 * H + BLOC], dt)
        nc.sync.dma_start(constf_sb[:], constf[:])
        w2st_sb = const.tile([128, HC * NPR], bf)
        nc.scalar.dma_start(w2st_sb[:], w2st[:])
        constp_sb = const.tile([NPR, H + 1], dt)
        nc.scalar.dma_start(constp_sb[:], constp[:])
        lp0b_sb = constf_sb[:, 0:H]
        db_sb = constf_sb[:, H : 2 * H]
        rbc_sb = constf_sb[:, 2 * H : 2 * H + BLOC]
        prom_sb = constp_sb[:, 0:H]
        c2_sb = constp_sb[:, H : H + 1]

        # Warm-up matmuls: sync the PE against the DVE memset and the w2st
        # DMA one dependency at a time — matmuls tolerate few sync waits.
        scrap_ps = warmp.tile([128, 1], dt)
        nc.tensor.matmul(scrap_ps[0:1, :], ones_bf[:], ones_bf[:],
                         start=True, stop=True)
        nc.tensor.matmul(scrap_ps[0:NPR, :], w2st_sb[:, 0:NPR], ones_bf[:],
                         start=True, stop=True)
        # preload the sigmoid table during boot (1.3us ACT_TABLE_LOAD that
        # would otherwise land on the critical tail)
        sig_warm = small.tile([1, 1], dt)
        nc.scalar.activation(sig_warm[:], ones_bf[0:1, 0:1],
                             func=mybir.ActivationFunctionType.Sigmoid)

        # column c*BLOC+b = hsumT chunk; trailing pad dim keeps each bf16
        # transpose output column on a 4-byte PSUM boundary
        hsumT_ps = sump.tile([128, HC * BLOC, 2], bf)

        for b in range(BLOC):
            if b < 2:
                t_in = t_ins[b]
            else:
                t_in = hidp.tile([128, LT, H], dt, tag="hid")
                # half-tile loads: downstream compute starts ~2us earlier
                # and the outbound stream interleaves more smoothly
                nc.sync.dma_start(t_in[:, 0:2], hid_r[b][:, 0:2])
                nc.sync.dma_start(t_in[:, 2:4], hid_r[b][:, 2:4])

            # bf16 shadow copy for the PE column sums (fp32 stationary loads
            # are ~6x slower and fp32 matmuls double-pump; the gate path
            # tolerates bf16 easily).  ACT, not GpSimd: the GpSimd CAST runs
            # ~5x slower and steals SBUF ports from the DVE.
            t_bf = bfp.tile([128, LT, H], bf, tag="hidbf")
            nc.scalar.copy(t_bf[:, 0:2], t_in[:, 0:2])
            nc.scalar.copy(t_bf[:, 2:4], t_in[:, 2:4])

            # stage 1a: hsum (1, 768) = sum over (t, p) via ones-stationary
            # matmuls (stationary loaded once, moving = bf16 rows; PSUM
            # accumulation over the 4 L-tiles; split 512/256 on bank edge)
            hs_ps = hsp.tile([1, H], dt, tag="hs")
            for t in range(LT):
                nc.tensor.matmul(hs_ps[0:1, 0:512], ones_bf[:],
                                 t_bf[:, t, 0:512],
                                 start=(t == 0), stop=(t == LT - 1))
                nc.tensor.matmul(hs_ps[0:1, 512:H], ones_bf[:],
                                 t_bf[:, t, 512:H],
                                 start=(t == 0), stop=(t == LT - 1))

            # stage 1b: downcast hsum, transpose 128-chunks onto partitions
            hs_bf = hsbp.tile([1, H], bf, tag="hsbf")
            nc.scalar.copy(hs_bf[:], hs_ps[:])
            for c in range(HC):
                col = c * BLOC + b
                nc.tensor.transpose(
                    hsumT_ps[:, col, 0:1],
                    hs_bf[0:1, c * 128 : (c + 1) * 128],
                    ones_bf[0:1, 0:1],
                )

            # per-batch gate pipeline — score columns are independent, so
            # gate/doc/DMA for batch b complete long before the kernel tail
            hsT_b = hsbp.tile([128, HC], bf, tag="hstb")
            nc.scalar.copy(hsT_b[:], hsumT_ps[:, b :: BLOC, 0])
            score_b = scop.tile([NPR, 1], dt, tag="scoreb")
            for c in range(HC):
                nc.tensor.matmul(
                    score_b[:], w2st_sb[:, c * NPR : (c + 1) * NPR],
                    hsT_b[:, c : c + 1],
                    start=(c == 0), stop=(c == HC - 1),
                )
            gate_b = hsbp.tile([NPR, 1], dt, tag="gateb")
            nc.scalar.activation(
                gate_b[:], score_b[:],
                func=mybir.ActivationFunctionType.Sigmoid,
                bias=c2_sb, scale=1.0,
            )
            doc_b = hsbp.tile([NPR, H], dt, tag="docb")
            nc.vector.tensor_scalar(
                doc_b[:], prom_sb, gate_b[:, 0:1], None,
                mybir.AluOpType.mult,
            )
            nc.scalar.dma_start(out[b, 0:NPR, :], doc_b[:])

            # rel[b] = lp0 + r_b * (lp1 - lp0), already partition-broadcast on
            # the host; r_b enters as a per-partition scalar (DVE one op).
            rel_t = relp.tile([128, H], dt, tag="relsb")
            nc.vector.scalar_tensor_tensor(
                rel_t[:], db_sb, rbc_sb[:, b : b + 1], lp0b_sb,
                mybir.AluOpType.mult, ADD,
            )

            # body = hid + rel (in place, free-dim broadcast of rel over
            # L-tiles), in halves so each outbound half-DMA starts as soon as
            # its add lands.  out-DMAs ride the ACT HWDGE ring so they don't
            # queue behind the next batch's in-DMA on the SP ring.
            for hlf in range(2):
                sl = slice(2 * hlf, 2 * hlf + 2)
                nc.vector.tensor_tensor(
                    t_in[:, sl], t_in[:, sl],
                    rel_t[:, None, :].broadcast_to([128, 2, H]),
                    ADD,
                )
                nc.scalar.dma_start(body_r[b][:, sl], t_in[:, sl])

    nc.compile()
    return nc


def _host_fold(relevance, prompts, label_prompts, qw, qb, kw, kb):
    """Fold the tiny projection weights on the host.

    scores[b, n] = hsum[b] . W2s[:, n] + c2[n], with W2s/c2 absorbing the
    1/L mean pooling and the 1/sqrt(HEAD) scaling.
    """
    q = prompts.astype(np.float64) @ qw.astype(np.float64).T + qb.astype(np.float64)
    w2 = q @ kw.astype(np.float64)                               # (10, H)
    w2s = (w2.T / (L * np.sqrt(HEAD))).astype(np.float32)        # (H, 10)
    c2 = ((q @ kb.astype(np.float64)) / np.sqrt(HEAD)).astype(np.float32)  # (10,)
    # device layout: (128, HC*NPR), free index = c*NPR + n for h = c*128 + p
    import ml_dtypes

    w2st = np.ascontiguousarray(
        w2s.reshape(HC, 128, NPR).transpose(1, 0, 2).reshape(128, HC * NPR)
    ).astype(ml_dtypes.bfloat16)
    return w2st, c2.reshape(NPR, 1)


def _prepare_in_maps(
    relevance, hidden_states_src, prompts, label_prompts,
    ref_qw, ref_qb, ref_kw, ref_kb, **_unused,
):
    relevance = np.asarray(relevance, dtype=np.float32)
    hidden_states_src = np.ascontiguousarray(np.asarray(hidden_states_src, dtype=np.float32))
    prompts = np.ascontiguousarray(np.asarray(prompts, dtype=np.float32))
    label_prompts = np.asarray(label_prompts, dtype=np.float32)

    w2st, c2 = _host_fold(
        relevance, prompts, label_prompts,
        np.asarray(ref_qw, np.float32), np.asarray(ref_qb, np.float32),
        np.asarray(ref_kw, np.float32), np.asarray(ref_kb, np.float32),
    )
    dvec = label_prompts[1] - label_prompts[0]
    constp = np.concatenate([prompts, c2.reshape(NPR, 1)], axis=1)  # (10, 769)

    in_maps = []
    for core in range(NCORES):
        sl = slice(core * BLOC, (core + 1) * BLOC)
        constf = np.empty((128, 2 * H + BLOC), np.float32)
        constf[:, 0:H] = label_prompts[0]
        constf[:, H : 2 * H] = dvec
        constf[:, 2 * H :] = relevance[sl]
        in_maps.append(
            {
                "hid": np.ascontiguousarray(hidden_states_src[sl]),
                "constf": constf,
                "constp": np.ascontiguousarray(constp),
                "w2st": w2st,
            }
        )
    return in_maps


def _get_module():
    if "nc" not in _CACHE:
        _CACHE["nc"] = _build_module()
    return _CACHE["nc"]


def kernel(**inputs):
    from concourse.bass_utils import run_bass_kernel_spmd

    nc = _get_module()
    in_maps = _prepare_in_maps(**inputs)
    res = run_bass_kernel_spmd(nc, in_maps, list(range(NCORES)))
    return np.concatenate([res.results[c]["out"] for c in range(NCORES)], axis=0)


# revision 62
# speedup vs baseline: 1.0394x; 1.0304x over previous
"""Bass/Trainium2 kernel for nn_DocRelPrompt.

Reference computation (B=64, L=512, H=768, HEAD=64, N_PROMPTS=10, N_LBL=2):
    rel2 = stack([1-r, r], 1)                   # (B, 2)
    hidden_rel = rel2 @ label_prompts           # (B, H)
    Q  = prompts @ ref_qw.T + ref_qb            # (10, HEAD)
    K  = hid @ ref_kw.T + ref_kb                # (B, L, HEAD)
    scores[b,n] = mean_l(Q[n] . K[b,l]) / 8
                = (hsum[b] . (Q@ref_kw)[n] / (512*8)) + (Q[n].ref_kb)/8
    gate = sigmoid(scores)                      # (B, 10)
    doc  = prompts[None] * gate[..., None]      # (B, 10, H)
    out  = concat([doc, hid + hidden_rel[:,None,:]], axis=1)   # (B, 522, H)

(The `_rel_prompts` branch of the reference is computed but unused, so it is
skipped entirely.)

Sharding: pure data-parallel over batch, 8 cores x 8 batches.  The tiny
prompt/weight tensors are folded on the host into W2s (768,10, bf16) and
c2 (10,) and replicated; the label-prompt vectors arrive partition-broadcast
so each per-batch hidden_rel row is a single DVE op.

Device work per core, per batch (DMA-bound; ~26 MB HBM traffic total):
  - two half-tile hid loads (128, 2, 768) on the SP HWDGE ring (the first
    two batches are issued ahead of the const loads — the ring is FIFO);
  - ACT casts a bf16 shadow (fp32 stationaries load ~6x slower and fp32
    matmuls double-pump; the sigmoid-gate path tolerates bf16 easily);
  - PE: hsum[b] (1, 768) via ones-stationary matmuls PSUM-accumulated over
    L-tiles, ACT downcast, then 6 PE transposes build hsumT columns (bf16,
    pad dim keeps each column 4-byte aligned in PSUM);
  - per-batch gate tail: 6 bf16 matmuls accumulate scores[:, b], ACT
    sigmoid(+c2), DVE tensor_scalar doc row, 30 KB doc DMA — all pipelined
    so only the last batch's chain trails the stream;
  - DVE: rel row = db*r_b + lp0b (scalar_tensor_tensor), then per half-tile
    an in-place tensor_tensor body = hid + rel and a 0.75 MB body DMA on
    the ACT HWDGE ring (so outs never queue behind the next in-load).
"""

import numpy as np

B, L, H, HEAD, NPR, NLBL = 64, 512, 768, 64, 10, 2
NCORES = 8
BLOC = B // NCORES          # 8 batches per core
LT = L // 128               # 4 L-tiles of 128 partitions
HC = H // 128               # 6 H-chunks of 128

_CACHE = {}


def _build_module():
    from contextlib import ExitStack

    import concourse.bacc as bacc
    import concourse.mybir as mybir
    from concourse.tile import TileContext

    dt = mybir.dt.float32
    bf = mybir.dt.bfloat16
    ADD = mybir.AluOpType.add

    # Bacc (not raw Bass): its compile() legalizes sync waits — TRN2
    # instructions carry at most one wait, extras become event-sem waits.
    nc = bacc.Bacc("TRN2", target_bir_lowering=False, debug=False)
    hid = nc.dram_tensor("hid", [BLOC, L, H], dt, kind="ExternalInput")
    # fp32 constants packed into two tensors: one DMA dispatch each (the SP
    # HWDGE ring is FIFO; every extra dispatch delays the hid stream ~0.65us)
    constf = nc.dram_tensor("constf", [128, 2 * H + BLOC], dt, kind="ExternalInput")
    constp = nc.dram_tensor("constp", [NPR, H + 1], dt, kind="ExternalInput")
    w2st = nc.dram_tensor("w2st", [128, HC * NPR], bf, kind="ExternalInput")
    out = nc.dram_tensor("out", [BLOC, NPR + L, H], dt, kind="ExternalOutput")

    hid_r = hid[:].rearrange("b (t p) h -> b p t h", p=128)
    body_r = out[:, NPR:, :].rearrange("b (t p) h -> b p t h", p=128)

    with TileContext(nc) as tc, ExitStack() as ctx:
        const = ctx.enter_context(tc.tile_pool(name="const", bufs=1))
        hidp = ctx.enter_context(tc.tile_pool(name="hidp", bufs=6))
        bfp = ctx.enter_context(tc.tile_pool(name="bfp", bufs=4))
        relp = ctx.enter_context(tc.tile_pool(name="relp", bufs=3))
        hsp = ctx.enter_context(tc.tile_pool(name="hsp", bufs=2, space="PSUM"))
        hsbp = ctx.enter_context(tc.tile_pool(name="hsbp", bufs=2))
        sump = ctx.enter_context(tc.tile_pool(name="sump", bufs=1, space="PSUM"))
        scop = ctx.enter_context(tc.tile_pool(name="scop", bufs=2, space="PSUM"))
        warmp = ctx.enter_context(tc.tile_pool(name="warmp", bufs=1, space="PSUM"))
        small = ctx.enter_context(tc.tile_pool(name="small", bufs=1))

        ones_bf = const.tile([128, 1], bf)
        nc.vector.memset(ones_bf[:], 1.0)

        # issue the first two hid loads BEFORE the const loads — the SP
        # HWDGE ring is FIFO, and const dispatches would otherwise delay the
        # first big transfer
        t_ins = []
        for b in range(2):
            t_in = hidp.tile([128, LT, H], dt, tag="hid")
            nc.sync.dma_start(t_in[:, 0:2], hid_r[b][:, 0:2])
            nc.sync.dma_start(t_in[:, 2:4], hid_r[b][:, 2:4])
            t_ins.append(t_in)

        # constf feeds the rel path (needed by the first body add) -> SP ring
        # right after the prefetches; w2st/constp are only needed by the
        # slack gate path -> ACT ring, which is empty this early
        constf_sb = const.tile([128, 2 * H + BLOC], dt)
        nc.sync.dma_start(constf_sb[:], constf[:])
        w2st_sb = const.tile([128, HC * NPR], bf)
        nc.scalar.dma_start(w2st_sb[:], w2st[:])
        constp_sb = const.tile([NPR, H + 1], dt)
        nc.scalar.dma_start(constp_sb[:], constp[:])
        lp0b_sb = constf_sb[:, 0:H]
        db_sb = constf_sb[:, H : 2 * H]
        rbc_sb = constf_sb[:, 2 * H : 2 * H + BLOC]
        prom_sb = constp_sb[:, 0:H]
        c2_sb = constp_sb[:, H : H + 1]

        # Warm-up matmuls: sync the PE against the DVE memset and the w2st
        # DMA one dependency at a time — matmuls tolerate few sync waits.
        scrap_ps = warmp.tile([128, 1], dt)
        nc.tensor.matmul(scrap_ps[0:1, :], ones_bf[:], ones_bf[:],
                         start=True, stop=True)
        nc.tensor.matmul(scrap_ps[0:NPR, :], w2st_sb[:, 0:NPR], ones_bf[:],
                         start=True, stop=True)
        # preload the sigmoid table during boot (1.3us ACT_TABLE_LOAD that
        # would otherwise land on the critical tail)
        sig_warm = small.tile([1, 1], dt)
        nc.scalar.activation(sig_warm[:], ones_bf[0:1, 0:1],
                             func=mybir.ActivationFunctionType.Sigmoid)

        # column c*BLOC+b = hsumT chunk; trailing pad dim keeps each bf16
        # transpose output column on a 4-byte PSUM boundary
        hsumT_ps = sump.tile([128, HC * BLOC, 2], bf)

        for b in range(BLOC):
            if b < 2:
                t_in = t_ins[b]
            else:
                t_in = hidp.tile([128, LT, H], dt, tag="hid")
                # half-tile loads: downstream compute starts ~2us earlier
                # and the outbound stream interleaves more smoothly
                nc.sync.dma_start(t_in[:, 0:2], hid_r[b][:, 0:2])
                nc.sync.dma_start(t_in[:, 2:4], hid_r[b][:, 2:4])

            # bf16 shadow copy for the PE column sums (fp32 stationary loads
            # are ~6x slower and fp32 matmuls double-pump; the gate path
            # tolerates bf16 easily).  ACT, not GpSimd: the GpSimd CAST runs
            # ~5x slower and steals SBUF ports from the DVE.
            t_bf = bfp.tile([128, LT, H], bf, tag="hidbf")
            nc.scalar.copy(t_bf[:, 0:2], t_in[:, 0:2])
            # second half on the DVE (single-src SBUF copy runs 2x mode);
            # ACT and PE otherwise co-pace the stream at ~80% busy
            nc.vector.tensor_copy(t_bf[:, 2:4], t_in[:, 2:4])

            # stage 1a: hsum (1, 768) = sum over (t, p) via ones-stationary
            # matmuls (stationary loaded once, moving = bf16 rows; PSUM
            # accumulation over the 4 L-tiles; split 512/256 on bank edge)
            hs_ps = hsp.tile([1, H], dt, tag="hs")
            for t in range(LT):
                nc.tensor.matmul(hs_ps[0:1, 0:512], ones_bf[:],
                                 t_bf[:, t, 0:512],
                                 start=(t == 0), stop=(t == LT - 1))
                nc.tensor.matmul(hs_ps[0:1, 512:H], ones_bf[:],
                                 t_bf[:, t, 512:H],
                                 start=(t == 0), stop=(t == LT - 1))

            # stage 1b: downcast hsum, transpose 128-chunks onto partitions
            hs_bf = hsbp.tile([1, H], bf, tag="hsbf")
            nc.scalar.copy(hs_bf[:], hs_ps[:])
            for c in range(HC):
                col = c * BLOC + b
                nc.tensor.transpose(
                    hsumT_ps[:, col, 0:1],
                    hs_bf[0:1, c * 128 : (c + 1) * 128],
                    ones_bf[0:1, 0:1],
                )

            # per-batch gate pipeline — score columns are independent, so
            # gate/doc/DMA for batch b complete long before the kernel tail
            hsT_b = hsbp.tile([128, HC], bf, tag="hstb")
            nc.scalar.copy(hsT_b[:], hsumT_ps[:, b :: BLOC, 0])
            score_b = scop.tile([NPR, 1], dt, tag="scoreb")
            for c in range(HC):
                nc.tensor.matmul(
                    score_b[:], w2st_sb[:, c * NPR : (c + 1) * NPR],
                    hsT_b[:, c : c + 1],
                    start=(c == 0), stop=(c == HC - 1),
                )
            gate_b = hsbp.tile([NPR, 1], dt, tag="gateb")
            nc.scalar.activation(
                gate_b[:], score_b[:],
                func=mybir.ActivationFunctionType.Sigmoid,
                bias=c2_sb, scale=1.0,
            )
            doc_b = hsbp.tile([NPR, H], dt, tag="docb")
            nc.vector.tensor_scalar(
                doc_b[:], prom_sb, gate_b[:, 0:1], None,
                mybir.AluOpType.mult,
            )
            nc.scalar.dma_start(out[b, 0:NPR, :], doc_b[:])

            # rel[b] = lp0 + r_b * (lp1 - lp0), already partition-broadcast on
            # the host; r_b enters as a per-partition scalar (DVE one op).
            rel_t = relp.tile([128, H], dt, tag="relsb")
            nc.vector.scalar_tensor_tensor(
                rel_t[:], db_sb, rbc_sb[:, b : b + 1], lp0b_sb,
                mybir.AluOpType.mult, ADD,
            )

            # body = hid + rel (in place, free-dim broadcast of rel over
            # L-tiles), in halves so each outbound half-DMA starts as soon as
            # its add lands.  out-DMAs ride the ACT HWDGE ring so they don't
            # queue behind the next batch's in-DMA on the SP ring.
            for hlf in range(2):
                sl = slice(2 * hlf, 2 * hlf + 2)
                nc.vector.tensor_tensor(
                    t_in[:, sl], t_in[:, sl],
                    rel_t[:, None, :].broadcast_to([128, 2, H]),
                    ADD,
                )
                nc.scalar.dma_start(body_r[b][:, sl], t_in[:, sl])

    nc.compile()
    return nc


def _host_fold(relevance, prompts, label_prompts, qw, qb, kw, kb):
    """Fold the tiny projection weights on the host.

    scores[b, n] = hsum[b] . W2s[:, n] + c2[n], with W2s/c2 absorbing the
    1/L mean pooling and the 1/sqrt(HEAD) scaling.
    """
    q = prompts.astype(np.float64) @ qw.astype(np.float64).T + qb.astype(np.float64)
    w2 = q @ kw.astype(np.float64)                               # (10, H)
    w2s = (w2.T / (L * np.sqrt(HEAD))).astype(np.float32)        # (H, 10)
    c2 = ((q @ kb.astype(np.float64)) / np.sqrt(HEAD)).astype(np.float32)  # (10,)
    # device layout: (128, HC*NPR), free index = c*NPR + n for h = c*128 + p
    import ml_dtypes

    w2st = np.ascontiguousarray(
        w2s.reshape(HC, 128, NPR).transpose(1, 0, 2).reshape(128, HC * NPR)
    ).astype(ml_dtypes.bfloat16)
    return w2st, c2.reshape(NPR, 1)


def _prepare_in_maps(
    relevance, hidden_states_src, prompts, label_prompts,
    ref_qw, ref_qb, ref_kw, ref_kb, **_unused,
):
    relevance = np.asarray(relevance, dtype=np.float32)
    hidden_states_src = np.ascontiguousarray(np.asarray(hidden_states_src, dtype=np.float32))
    prompts = np.ascontiguousarray(np.asarray(prompts, dtype=np.float32))
    label_prompts = np.asarray(label_prompts, dtype=np.float32)

    w2st, c2 = _host_fold(
        relevance, prompts, label_prompts,
        np.asarray(ref_qw, np.float32), np.asarray(ref_qb, np.float32),
        np.asarray(ref_kw, np.float32), np.asarray(ref_kb, np.float32),
    )
    dvec = label_prompts[1] - label_prompts[0]
    constp = np.concatenate([prompts, c2.reshape(NPR, 1)], axis=1)  # (10, 769)

    in_maps = []
    for core in range(NCORES):
        sl = slice(core * BLOC, (core + 1) * BLOC)
        constf = np.empty((128, 2 * H + BLOC), np.float32)
        constf[:, 0:H] = label_prompts[0]
        constf[:, H : 2 * H] = dvec
        constf[:, 2 * H :] = relevance[sl]
        in_maps.append(
            {
                "hid": np.ascontiguousarray(hidden_states_src[sl]),
                "constf": constf,
                "constp": np.ascontiguousarray(constp),
                "w2st": w2st,
            }
        )
    return in_maps


def _get_module():
    if "nc" not in _CACHE:
        _CACHE["nc"] = _build_module()
    return _CACHE["nc"]


def kernel(**inputs):
    from concourse.bass_utils import run_bass_kernel_spmd

    nc = _get_module()
    in_maps = _prepare_in_maps(**inputs)
    res = run_bass_kernel_spmd(nc, in_maps, list(range(NCORES)))
    return np.concatenate([res.results[c]["out"] for c in range(NCORES)], axis=0)


# revision 63
# speedup vs baseline: 1.0518x; 1.0119x over previous
"""Bass/Trainium2 kernel for nn_DocRelPrompt.

Reference computation (B=64, L=512, H=768, HEAD=64, N_PROMPTS=10, N_LBL=2):
    rel2 = stack([1-r, r], 1)                   # (B, 2)
    hidden_rel = rel2 @ label_prompts           # (B, H)
    Q  = prompts @ ref_qw.T + ref_qb            # (10, HEAD)
    K  = hid @ ref_kw.T + ref_kb                # (B, L, HEAD)
    scores[b,n] = mean_l(Q[n] . K[b,l]) / 8
                = (hsum[b] . (Q@ref_kw)[n] / (512*8)) + (Q[n].ref_kb)/8
    gate = sigmoid(scores)                      # (B, 10)
    doc  = prompts[None] * gate[..., None]      # (B, 10, H)
    out  = concat([doc, hid + hidden_rel[:,None,:]], axis=1)   # (B, 522, H)

(The `_rel_prompts` branch of the reference is computed but unused, so it is
skipped entirely.)

Sharding: pure data-parallel over batch, 8 cores x 8 batches.  The tiny
prompt/weight tensors are folded on the host into W2s (768,10, bf16) and
c2 (10,) and replicated; the label-prompt vectors arrive partition-broadcast
so each per-batch hidden_rel row is a single DVE op.

Device work per core, per batch (DMA-bound; ~26 MB HBM traffic total):
  - two half-tile hid loads (128, 2, 768) on the SP HWDGE ring (the first
    two batches are issued ahead of the const loads — the ring is FIFO);
  - ACT casts a bf16 shadow (fp32 stationaries load ~6x slower and fp32
    matmuls double-pump; the sigmoid-gate path tolerates bf16 easily);
  - PE: hsum[b] (1, 768) via ones-stationary matmuls PSUM-accumulated over
    L-tiles, ACT downcast, then 6 PE transposes build hsumT columns (bf16,
    pad dim keeps each column 4-byte aligned in PSUM);
  - per-batch gate tail: 6 bf16 matmuls accumulate scores[:, b], ACT
    sigmoid(+c2), DVE tensor_scalar doc row, 30 KB doc DMA — all pipelined
    so only the last batch's chain trails the stream;
  - DVE: rel row = db*r_b + lp0b (scalar_tensor_tensor), then per half-tile
    an in-place tensor_tensor body = hid + rel and a 0.75 MB body DMA on
    the ACT HWDGE ring (so outs never queue behind the next in-load).
"""

import numpy as np

B, L, H, HEAD, NPR, NLBL = 64, 512, 768, 64, 10, 2
NCORES = 8
BLOC = B // NCORES          # 8 batches per core
LT = L // 128               # 4 L-tiles of 128 partitions
HC = H // 128               # 6 H-chunks of 128

_CACHE = {}


def _build_module():
    from contextlib import ExitStack

    import concourse.bacc as bacc
    import concourse.mybir as mybir
    from concourse.tile import TileContext

    dt = mybir.dt.float32
    bf = mybir.dt.bfloat16
    ADD = mybir.AluOpType.add

    # Bacc (not raw Bass): its compile() legalizes sync waits — TRN2
    # instructions carry at most one wait, extras become event-sem waits.
    nc = bacc.Bacc("TRN2", target_bir_lowering=False, debug=False)
    hid = nc.dram_tensor("hid", [BLOC, L, H], dt, kind="ExternalInput")
    # fp32 constants packed into two tensors: one DMA dispatch each (the SP
    # HWDGE ring is FIFO; every extra dispatch delays the hid stream ~0.65us)
    constf = nc.dram_tensor("constf", [128, 2 * H + BLOC], dt, kind="ExternalInput")
    constp = nc.dram_tensor("constp", [NPR, H + 1], dt, kind="ExternalInput")
    w2st = nc.dram_tensor("w2st", [128, HC * NPR], bf, kind="ExternalInput")
    out = nc.dram_tensor("out", [BLOC, NPR + L, H], dt, kind="ExternalOutput")

    hid_r = hid[:].rearrange("b (t p) h -> b p t h", p=128)
    body_r = out[:, NPR:, :].rearrange("b (t p) h -> b p t h", p=128)

    with TileContext(nc) as tc, ExitStack() as ctx:
        const = ctx.enter_context(tc.tile_pool(name="const", bufs=1))
        hidp = ctx.enter_context(tc.tile_pool(name="hidp", bufs=6))
        bfp = ctx.enter_context(tc.tile_pool(name="bfp", bufs=4))
        relp = ctx.enter_context(tc.tile_pool(name="relp", bufs=3))
        hsp = ctx.enter_context(tc.tile_pool(name="hsp", bufs=2, space="PSUM"))
        hsbp = ctx.enter_context(tc.tile_pool(name="hsbp", bufs=2))
        sump = ctx.enter_context(tc.tile_pool(name="sump", bufs=1, space="PSUM"))
        scop = ctx.enter_context(tc.tile_pool(name="scop", bufs=2, space="PSUM"))
        warmp = ctx.enter_context(tc.tile_pool(name="warmp", bufs=1, space="PSUM"))
        small = ctx.enter_context(tc.tile_pool(name="small", bufs=1))

        ones_bf = const.tile([128, 1], bf)
        nc.vector.memset(ones_bf[:], 1.0)

        # issue the first two hid loads BEFORE the const loads — the SP
        # HWDGE ring is FIFO, and const dispatches would otherwise delay the
        # first big transfer
        t_ins = []
        for b in range(2):
            t_in = hidp.tile([128, LT, H], dt, tag="hid")
            nc.sync.dma_start(t_in[:, 0:2], hid_r[b][:, 0:2])
            nc.sync.dma_start(t_in[:, 2:4], hid_r[b][:, 2:4])
            t_ins.append(t_in)

        # constf feeds the rel path (needed by the first body add) -> SP ring
        # right after the prefetches; w2st/constp are only needed by the
        # slack gate path -> ACT ring, which is empty this early
        constf_sb = const.tile([128, 2 * H + BLOC], dt)
        nc.sync.dma_start(constf_sb[:], constf[:])
        w2st_sb = const.tile([128, HC * NPR], bf)
        nc.scalar.dma_start(w2st_sb[:], w2st[:])
        constp_sb = const.tile([NPR, H + 1], dt)
        nc.scalar.dma_start(constp_sb[:], constp[:])
        lp0b_sb = constf_sb[:, 0:H]
        db_sb = constf_sb[:, H : 2 * H]
        rbc_sb = constf_sb[:, 2 * H : 2 * H + BLOC]
        prom_sb = constp_sb[:, 0:H]
        c2_sb = constp_sb[:, H : H + 1]

        # Warm-up matmuls: sync the PE against the DVE memset and the w2st
        # DMA one dependency at a time — matmuls tolerate few sync waits.
        scrap_ps = warmp.tile([128, 1], dt)
        nc.tensor.matmul(scrap_ps[0:1, :], ones_bf[:], ones_bf[:],
                         start=True, stop=True)
        nc.tensor.matmul(scrap_ps[0:NPR, :], w2st_sb[:, 0:NPR], ones_bf[:],
                         start=True, stop=True)
        # preload the sigmoid table during boot (1.3us ACT_TABLE_LOAD that
        # would otherwise land on the critical tail)
        sig_warm = small.tile([1, 1], dt)
        nc.scalar.activation(sig_warm[:], ones_bf[0:1, 0:1],
                             func=mybir.ActivationFunctionType.Sigmoid)

        # column c*BLOC+b = hsumT chunk; trailing pad dim keeps each bf16
        # transpose output column on a 4-byte PSUM boundary
        hsumT_ps = sump.tile([128, HC * BLOC, 2], bf)

        for b in range(BLOC):
            if b < 2:
                t_in = t_ins[b]
            else:
                t_in = hidp.tile([128, LT, H], dt, tag="hid")
                # half-tile loads: downstream compute starts ~2us earlier
                # and the outbound stream interleaves more smoothly
                nc.sync.dma_start(t_in[:, 0:2], hid_r[b][:, 0:2])
                nc.sync.dma_start(t_in[:, 2:4], hid_r[b][:, 2:4])

            # bf16 shadow copy for the PE column sums (fp32 stationary loads
            # are ~6x slower and fp32 matmuls double-pump; the gate path
            # tolerates bf16 easily).  ACT, not GpSimd: the GpSimd CAST runs
            # ~5x slower and steals SBUF ports from the DVE.
            t_bf = bfp.tile([128, LT, H], bf, tag="hidbf")
            nc.scalar.copy(t_bf[:, 0:2], t_in[:, 0:2])
            # second half on the DVE (single-src SBUF copy runs 2x mode);
            # ACT and PE otherwise co-pace the stream at ~80% busy
            nc.vector.tensor_copy(t_bf[:, 2:4], t_in[:, 2:4])

            # stage 1a: hsum (1, 768) = sum over (t, p) via ones-stationary
            # matmuls (stationary loaded once, moving = bf16 rows; PSUM
            # accumulation over the 4 L-tiles; split 512/256 on bank edge)
            hs_ps = hsp.tile([1, H], dt, tag="hs")
            for t in range(LT):
                nc.tensor.matmul(hs_ps[0:1, 0:512], ones_bf[:],
                                 t_bf[:, t, 0:512],
                                 start=(t == 0), stop=(t == LT - 1))
                nc.tensor.matmul(hs_ps[0:1, 512:H], ones_bf[:],
                                 t_bf[:, t, 512:H],
                                 start=(t == 0), stop=(t == LT - 1))

            # stage 1b: downcast hsum, transpose 128-chunks onto partitions
            hs_bf = hsbp.tile([1, H], bf, tag="hsbf")
            nc.scalar.copy(hs_bf[:], hs_ps[:])
            for c in range(HC):
                col = c * BLOC + b
                nc.tensor.transpose(
                    hsumT_ps[:, col, 0:1],
                    hs_bf[0:1, c * 128 : (c + 1) * 128],
                    ones_bf[0:1, 0:1],
                )

            # gate pipeline per batch PAIR — score columns are independent;
            # pairing halves the tiny stage-2 matmuls / sigmoids / doc DMAs
            # on the 84%-busy PE and ACT engines
            if b % 2 == 0:
                hsT_p = hsbp.tile([128, HC, 2], bf, tag="hstp")
            nc.scalar.copy(hsT_p[:, :, b % 2], hsumT_ps[:, b :: BLOC, 0])
            if b % 2 == 1:
                score_p = scop.tile([NPR, 2], dt, tag="scorep")
                for c in range(HC):
                    nc.tensor.matmul(
                        score_p[:], w2st_sb[:, c * NPR : (c + 1) * NPR],
                        hsT_p[:, c, 0:2],
                        start=(c == 0), stop=(c == HC - 1),
                    )
                gate_p = hsbp.tile([NPR, 2], dt, tag="gatep")
                nc.scalar.activation(
                    gate_p[:], score_p[:],
                    func=mybir.ActivationFunctionType.Sigmoid,
                    bias=c2_sb, scale=1.0,
                )
                doc_p = hsbp.tile([NPR, 2, H], dt, tag="docp")
                for j in range(2):
                    nc.vector.tensor_scalar(
                        doc_p[:, j, :], prom_sb, gate_p[:, j : j + 1], None,
                        mybir.AluOpType.mult,
                    )
                nc.scalar.dma_start(
                    out[b - 1 : b + 1, 0:NPR, :].transpose([1, 0, 2]), doc_p[:]
                )

            # rel[b] = lp0 + r_b * (lp1 - lp0), already partition-broadcast on
            # the host; r_b enters as a per-partition scalar (DVE one op).
            rel_t = relp.tile([128, H], dt, tag="relsb")
            nc.vector.scalar_tensor_tensor(
                rel_t[:], db_sb, rbc_sb[:, b : b + 1], lp0b_sb,
                mybir.AluOpType.mult, ADD,
            )

            # body = hid + rel (in place, free-dim broadcast of rel over
            # L-tiles), in halves so each outbound half-DMA starts as soon as
            # its add lands.  out-DMAs ride the ACT HWDGE ring so they don't
            # queue behind the next batch's in-DMA on the SP ring.
            for hlf in range(2):
                sl = slice(2 * hlf, 2 * hlf + 2)
                nc.vector.tensor_tensor(
                    t_in[:, sl], t_in[:, sl],
                    rel_t[:, None, :].broadcast_to([128, 2, H]),
                    ADD,
                )
                nc.scalar.dma_start(body_r[b][:, sl], t_in[:, sl])

    nc.compile()
    return nc


def _host_fold(relevance, prompts, label_prompts, qw, qb, kw, kb):
    """Fold the tiny projection weights on the host.

    scores[b, n] = hsum[b] . W2s[:, n] + c2[n], with W2s/c2 absorbing the
    1/L mean pooling and the 1/sqrt(HEAD) scaling.
    """
    q = prompts.astype(np.float64) @ qw.astype(np.float64).T + qb.astype(np.float64)
    w2 = q @ kw.astype(np.float64)                               # (10, H)
    w2s = (w2.T / (L * np.sqrt(HEAD))).astype(np.float32)        # (H, 10)
    c2 = ((q @ kb.astype(np.float64)) / np.sqrt(HEAD)).astype(np.float32)  # (10,)
    # device layout: (128, HC*NPR), free index = c*NPR + n for h = c*128 + p
    import ml_dtypes

    w2st = np.ascontiguousarray(
        w2s.reshape(HC, 128, NPR).transpose(1, 0, 2).reshape(128, HC * NPR)
    ).astype(ml_dtypes.bfloat16)
    return w2st, c2.reshape(NPR, 1)


def _prepare_in_maps(
    relevance, hidden_states_src, prompts, label_prompts,
    ref_qw, ref_qb, ref_kw, ref_kb, **_unused,
):
    relevance = np.asarray(relevance, dtype=np.float32)
    hidden_states_src = np.ascontiguousarray(np.asarray(hidden_states_src, dtype=np.float32))
    prompts = np.ascontiguousarray(np.asarray(prompts, dtype=np.float32))
    label_prompts = np.asarray(label_prompts, dtype=np.float32)

    w2st, c2 = _host_fold(
        relevance, prompts, label_prompts,
        np.asarray(ref_qw, np.float32), np.asarray(ref_qb, np.float32),
        np.asarray(ref_kw, np.float32), np.asarray(ref_kb, np.float32),
    )
    dvec = label_prompts[1] - label_prompts[0]
    constp = np.concatenate([prompts, c2.reshape(NPR, 1)], axis=1)  # (10, 769)

    in_maps = []
    for core in range(NCORES):
        sl = slice(core * BLOC, (core + 1) * BLOC)
        constf = np.empty((128, 2 * H + BLOC), np.float32)
        constf[:, 0:H] = label_prompts[0]
        constf[:, H : 2 * H] = dvec
        constf[:, 2 * H :] = relevance[sl]
        in_maps.append(
            {
                "hid": np.ascontiguousarray(hidden_states_src[sl]),
                "constf": constf,
                "constp": np.ascontiguousarray(constp),
                "w2st": w2st,
            }
        )
    return in_maps


def _get_module():
    if "nc" not in _CACHE:
        _CACHE["nc"] = _build_module()
    return _CACHE["nc"]


def kernel(**inputs):
    from concourse.bass_utils import run_bass_kernel_spmd

    nc = _get_module()
    in_maps = _prepare_in_maps(**inputs)
    res = run_bass_kernel_spmd(nc, in_maps, list(range(NCORES)))
    return np.concatenate([res.results[c]["out"] for c in range(NCORES)], axis=0)
